# revision 1
# baseline (speedup 1.0000x reference)
"""Causal self-attention with RoPE on 8 Trainium2 NeuronCores.

Reference computation (B=4, T=2048, D=1024, H=16, hd=64, fp32):
    qkv = x @ w_qkv ; q,k per-head RoPE (interleaved pairs) ;
    out = softmax(causal(q k^T / 8)) @ v ; out @ w_proj

Sharding: core c -> (batch b = c//2, head-group g = c%2 of 8 heads).
Data parallel on B, tensor parallel on heads; w_proj is row-parallel so each
core returns a partial [2048, 1024] product and the host sums the two
partials per batch (the "all-reduce" of the row-parallel linear).

Per-core device program, all matmul inputs bf16 (full PE speed at any tile
size, fp32 PSUM accumulation), software-pipelined so PE never waits on the
softmax exp chain:

  1. x^T, weights, RoPE tables prepared host-side in bf16. The q/k weight
     columns are permuted per head into [quadrant][even|odd] order so the
     RoPE partner swap becomes a single DVE stream_shuffle (mask i^16 within
     each 32-partition quadrant); the rotation sign is folded into the sin
     table, the 1/sqrt(hd) score scale into the k columns.
  2. Per head-pair p: q^T,k^T = w_a^T @ x^T into PSUM; Pool copies PSUM->bf16;
     DVE shuffle + 2 mults + add apply RoPE into qkt tiles.  v in natural
     layout [t, feat] with a ones column appended per head (softmax
     denominators fall out of the PV matmul for free).
  3. Attention per (pair, i-half, head), scores transposed S^T[j, i]:
     per key tile jt: matmul segments into st PSUM, ACT exp -> bf16 P^T
     tiles, causal diagonal handled by a bf16 multiply with a 0/1 triangle,
     PV accumulated as matmul(lhsT=[v | ones], rhs=P^T) -> [65, 1024] PSUM.
     Normalize via reciprocal_approx_fast + Pool partition_broadcast + mult.
  4. out_partial = attn^T @ w_proj per token tile -> DRAM f32.

  Emission interleaves next-pair QKV (and later the projection) into the
  attention streams so the PE instruction queue always has matmuls ready
  while ACT/DVE/Pool work through exp/rope/normalize.
"""

import numpy as np
import ml_dtypes

import concourse.bass as bass
import concourse.tile as tile
from concourse import bacc, mybir
from concourse.bass_utils import run_bass_kernel_spmd

F32 = mybir.dt.float32
BF16 = mybir.dt.bfloat16
AF = mybir.ActivationFunctionType
OP = mybir.AluOpType

B, T, D, NH, HD = 4, 2048, 1024, 16, 64
HPC = 8            # heads per core
N_CORES = 8
NTT = T // 128     # 16 token tiles
NKT = D // 128     # 8 contraction tiles
SHUF_MASK = [i ^ 16 for i in range(32)]

BF = ml_dtypes.bfloat16


def _build_program(reps: int = 1):
    nc = bacc.Bacc("TRN2", target_bir_lowering=False, debug=False)
    x_d = nc.dram_tensor("x", [D, T], BF16, kind="ExternalInput")     # x^T
    wqk_d = nc.dram_tensor("wqk", [128, NKT, 1024], BF16, kind="ExternalInput")
    wv_d = nc.dram_tensor("wv", [128, NKT, 512], BF16, kind="ExternalInput")
    wp_d = nc.dram_tensor("wproj", [128, 4, D], BF16, kind="ExternalInput")
    cc_d = nc.dram_tensor("cc", [128, T], BF16, kind="ExternalInput")
    ss_d = nc.dram_tensor("ss", [128, T], BF16, kind="ExternalInput")
    tri_d = nc.dram_tensor("tri", [128, 128], BF16, kind="ExternalInput")
    out_d = nc.dram_tensor("out", [T, D], F32, kind="ExternalOutput")

    with tile.TileContext(nc) as tc:
      for _rep in range(reps):
        with (
            tc.tile_pool(name="persist", bufs=1) as pers,
            tc.tile_pool(name="vo", bufs=1) as vop,
            tc.tile_pool(name="qkt", bufs=1) as qktp,
            tc.tile_pool(name="at", bufs=1) as atp,
            tc.tile_pool(name="xt", bufs=1) as xtp,
            tc.tile_pool(name="ftw", bufs=4) as ftwp,
            tc.tile_pool(name="rope", bufs=3) as ropetp,
            tc.tile_pool(name="pt", bufs=4) as ptp,
            tc.tile_pool(name="nrm", bufs=2) as nrmp,
            tc.tile_pool(name="osb", bufs=2) as osbp,
        ):
            tri = pers.tile([128, 128], BF16, tag="tri")
            cc = pers.tile([128, T], BF16, tag="cc")
            ss = pers.tile([128, T], BF16, tag="ss")
            wv_sb = pers.tile([128, NKT, 512], BF16, tag="wv")
            wp_sb = pers.tile([128, 4, D], BF16, tag="wp")
            xt = [xtp.tile([128, T], BF16, tag=f"xt{kt}", name=f"xt{kt}")
                  for kt in range(NKT)]

            qkt = [qktp.tile([128, T], BF16, tag=f"qkt{i}", name=f"qkt{i}")
                   for i in range(8)]
            vo = [vop.tile([128, HPC, 65], BF16, tag=f"vo{tt}", name=f"vo{tt}")
                  for tt in range(NTT)]
            at = [atp.tile([128, T], BF16, tag=f"at{pr}", name=f"at{pr}")
                  for pr in range(4)]

            # warm the ACT exp table before it's on the critical path
            warm = ropetp.tile([1, 8], F32, tag="warm")
            nc.vector.memset(warm[:], 0.0)
            nc.scalar.activation(warm[:], warm[:], AF.Exp)

            def emit_rope(ps_a, ft, tcn):
                # u = q * ssn_pre (row-permuted sin table), so
                # shuffle(u)[p] = q[p^16] * ssn[p] == the RoPE partner
                # term; shuffle stays bf16->bf16.
                sl = slice(tcn * 512, (tcn + 1) * 512)
                u = ropetp.tile([128, 512], BF16, tag="u",
                                name=f"u{ft}_{tcn}")
                nc.vector.tensor_tensor(u[:], ps_a[:], ss[:, sl], OP.mult)
                t1 = ropetp.tile([128, 512], BF16, tag="t1")
                t2 = ropetp.tile([128, 512], BF16, tag="t2")
                nc.vector.tensor_tensor(t1[:], ps_a[:], cc[:, sl], OP.mult)
                nc.vector.stream_shuffle(t2[:], u[:], SHUF_MASK)
                nc.vector.tensor_tensor(qkt[ft][:, sl], t1[:], t2[:], OP.add)

            # ---- pair 0: kt-outer with 8 PSUM accumulators so the PE
            # starts as soon as the first x tile lands (x DMA ~12us).
            wa0 = {}
            for side in range(2):
                ft = 4 * side
                wa0[ft] = ftwp.tile([128, NKT, 128], BF16, tag="wa",
                                    name=f"wa{ft}")
                nc.sync.dma_start(wa0[ft][:],
                                  wqk_d[:, :, ft * 128:(ft + 1) * 128])
            for kt in range(2):
                nc.sync.dma_start(xt[kt][:], x_d[kt * 128:(kt + 1) * 128, :])
            nc.sync.dma_start(cc[:], cc_d[:])
            nc.sync.dma_start(ss[:], ss_d[:])
            for kt in range(2, NKT):
                nc.sync.dma_start(xt[kt][:], x_d[kt * 128:(kt + 1) * 128, :])
            nc.sync.dma_start(tri[:], tri_d[:])
            nc.sync.dma_start(wv_sb[:], wv_d[:])
            nc.sync.dma_start(wp_sb[:], wp_d[:])

            with tc.tile_pool(name="qk0ps", bufs=1, space="PSUM") as qk0ps:
                acc = {}
                for side in range(2):
                    for tcn in range(4):
                        acc[(side, tcn)] = qk0ps.tile(
                            [128, 512], F32, tag=f"a{side}{tcn}",
                            name=f"qk0a{side}_{tcn}")
                for side in range(2):
                    for tcn in range(4):
                        sl = slice(tcn * 512, (tcn + 1) * 512)
                        for kt in range(NKT):
                            nc.tensor.matmul(
                                acc[(side, tcn)][:], wa0[4 * side][:, kt, :],
                                xt[kt][:, sl],
                                start=(kt == 0), stop=(kt == NKT - 1),
                            )
                for side in range(2):
                    for tcn in range(4):
                        emit_rope(acc[(side, tcn)], 4 * side, tcn)

            with (
                tc.tile_pool(name="qkps", bufs=2, space="PSUM") as qkps,
            ):
                # ---- QKV emission steps (pairs 1..3) -------------------
                def qk_steps(pr):
                    steps = []
                    for side in range(2):          # 0 = q, 1 = k
                        ft = 4 * side + pr
                        w_a = ftwp.tile([128, NKT, 128], BF16, tag="wa",
                                        name=f"wa{ft}")
                        def load_w(ft=ft, w_a=w_a):
                            nc.sync.dma_start(
                                w_a[:], wqk_d[:, :, ft * 128:(ft + 1) * 128])
                        steps.append(load_w)
                        for tcn in range(4):
                            def chunk(ft=ft, w_a=w_a, tcn=tcn):
                                sl = slice(tcn * 512, (tcn + 1) * 512)
                                ps_a = qkps.tile([128, 512], F32, tag="ps",
                                                 name=f"qk{ft}_{tcn}")
                                for kt in range(NKT):
                                    nc.tensor.matmul(
                                        ps_a[:], w_a[:, kt, :], xt[kt][:, sl],
                                        start=(kt == 0), stop=(kt == NKT - 1),
                                    )
                                emit_rope(ps_a, ft, tcn)
                            steps.append(chunk)
                    return steps

                def v_steps():
                    steps = []
                    with tc.tile_pool(name="vps", bufs=2, space="PSUM") as vps:
                        for tt in range(NTT):
                            def vchunk(tt=tt):
                                nc.gpsimd.memset(vo[tt][:], 1.0)
                                ps = vps.tile([128, 512], F32, tag="vps")
                                for kt in range(NKT):
                                    nc.tensor.matmul(
                                        ps[:],
                                        xt[kt][:, tt * 128:(tt + 1) * 128],
                                        wv_sb[:, kt, :],
                                        start=(kt == 0), stop=(kt == NKT - 1),
                                    )
                                nc.scalar.copy(
                                    vo[tt][:, :, 0:64],
                                    ps[:].rearrange("p (h d) -> p h d", h=HPC),
                                )
                            steps.append(vchunk)
                        # vps pool must close after all emissions; execute here
                        for s in steps:
                            s()
                    return []

                v_steps()

                with (
                    tc.tile_pool(name="stps", bufs=2, space="PSUM") as stps,
                    tc.tile_pool(name="atps", bufs=1, space="PSUM") as atps,
                ):
                    def attn_steps(pr, ih):
                        """Per (pair, i-half): two heads sequentially."""
                        i0 = 1024 * ih
                        n_jt = 8 * ih + 8
                        steps = []

                        def jt_segs(jt):
                            j0 = 128 * jt
                            i_lo = max(i0, j0)
                            segs = []
                            lo = i_lo
                            while lo < i0 + 1024:
                                hi = min(i0 + 1024, (lo // 512 + 1) * 512)
                                segs.append((lo, hi))
                                lo = hi
                            return j0, i_lo, segs

                        for h in (2 * pr, 2 * pr + 1):
                            r0 = 64 * (h % 2)
                            # allocated lazily at emission time: the bufs=1
                            # slot's WAR ordering follows instruction order
                            cell = {}

                            def get_at_ps(cell=cell, ih=ih, h=h):
                                if "t" not in cell:
                                    cell["t"] = atps.tile(
                                        [65, 1024], F32, tag="atps",
                                        name=f"at{ih}_{h}")
                                return cell["t"]

                            def emit_st(jt, h=h, r0=r0):
                                qt_ap = qkt[pr][r0:r0 + 64, :]
                                kt_ap = qkt[4 + pr][r0:r0 + 64, :]
                                j0, i_lo, segs = jt_segs(jt)
                                st = stps.tile([128, 1024], F32, tag="st",
                                               name=f"st{ih}_{h}_{jt}")
                                for (lo, hi) in segs:
                                    nc.tensor.matmul(
                                        st[:, lo - i0:hi - i0],
                                        kt_ap[:, j0:j0 + 128],
                                        qt_ap[:, lo:hi],
                                        start=True, stop=True,
                                    )
                                pt = ptp.tile([128, 1024], BF16, tag="pt",
                                              name=f"pt{ih}_{h}_{jt}")
                                w = i0 + 1024 - i_lo
                                nc.scalar.activation(
                                    pt[:, 0:w], st[:, i_lo - i0:], AF.Exp)
                                if j0 >= i0:
                                    nc.vector.tensor_tensor(
                                        pt[:, 0:128], pt[:, 0:128], tri[:],
                                        OP.mult)
                                return pt

                            def emit_pv(jt, pt, h=h, get_at_ps=get_at_ps):
                                at_ps = get_at_ps()
                                j0, i_lo, segs = jt_segs(jt)
                                for (lo, hi) in segs:
                                    last_jt = min(n_jt - 1, (hi - 1) // 128)
                                    nc.tensor.matmul(
                                        at_ps[:, lo - i0:hi - i0],
                                        vo[jt][:, h, :],
                                        pt[:, lo - i_lo:hi - i_lo],
                                        start=(jt == 0), stop=(jt == last_jt),
                                    )

                            def normalize(h=h, r0=r0, get_at_ps=get_at_ps):
                                at_ps = get_at_ps()
                                sum_sb = nrmp.tile([1, 1024], F32, tag="sum")
                                r_sb = nrmp.tile([1, 1024], F32, tag="r")
                                rb_sb = nrmp.tile([64, 1024], F32, tag="rb")
                                nc.vector.tensor_copy(sum_sb[:], at_ps[64:65, :])
                                nc.vector.reciprocal_approx_fast(r_sb[:], sum_sb[:])
                                nc.gpsimd.partition_broadcast(rb_sb[:], r_sb[:])
                                nc.vector.tensor_tensor(
                                    at[pr][r0:r0 + 64, i0:i0 + 1024],
                                    at_ps[0:64, :], rb_sb[:], OP.mult)

                            def step(jt, emit_st=emit_st, emit_pv=emit_pv,
                                     normalize=normalize,
                                     state={"prev": None}):
                                pt = emit_st(jt)
                                if state["prev"] is not None:
                                    emit_pv(jt - 1, state["prev"])
                                state["prev"] = pt
                                if jt == n_jt - 1:
                                    emit_pv(jt, pt)
                                    normalize()

                            for jt in range(n_jt):
                                steps.append(lambda jt=jt, step=step: step(jt))
                        return steps

                    def proj_steps(tt_range):
                        steps = []
                        for tt in tt_range:
                            def ptt(tt=tt):
                                o_sb = osbp.tile([128, D], F32, tag="osb")
                                for nch in range(2):
                                    pp = qkps.tile([128, 512], F32, tag="ps",
                                                   name=f"pp{tt}_{nch}")
                                    for mt in range(4):
                                        nc.tensor.matmul(
                                            pp[:],
                                            at[mt][:, tt * 128:(tt + 1) * 128],
                                            wp_sb[:, mt, nch * 512:(nch + 1) * 512],
                                            start=(mt == 0), stop=(mt == 3),
                                        )
                                    nc.vector.tensor_copy(
                                        o_sb[:, nch * 512:(nch + 1) * 512], pp[:])
                                nc.sync.dma_start(
                                    out_d[tt * 128:(tt + 1) * 128, :], o_sb[:])
                            steps.append(ptt)
                        return steps

                    def merge(a_steps, b_steps):
                        out = []
                        na, nb = len(a_steps), len(b_steps)
                        j = 0
                        for i, s in enumerate(a_steps):
                            out.append(s)
                            while j * na < (i + 1) * nb:
                                out.append(b_steps[j])
                                j += 1
                        out.extend(b_steps[j:])
                        return out

                    # ih=0 phase: attention(p) with next pair's qkv merged in
                    for pr in range(4):
                        a = attn_steps(pr, 0)
                        b = qk_steps(pr + 1) if pr < 3 else []
                        for s in merge(a, b):
                            s()
                    # ih=1 phase: attention(p) with first-half proj merged in
                    for pr in range(4):
                        a = attn_steps(pr, 1)
                        b = proj_steps(range(2 * pr, 2 * pr + 2))
                        for s in merge(a, b):
                            s()
                    for s in proj_steps(range(8, 16)):
                        s()
    nc.compile()
    return nc


_NC_CACHE = None


def _get_program():
    global _NC_CACHE
    if _NC_CACHE is None:
        _NC_CACHE = _build_program()
    return _NC_CACHE


def _host_inputs(x, cos, sin, w_qkv, w_proj):
    """Build the 8 per-core input dicts (bf16, pre-permuted layouts)."""
    x = np.asarray(x, np.float32)
    cos = np.asarray(cos, np.float32)
    sin = np.asarray(sin, np.float32)
    w_qkv = np.asarray(w_qkv, np.float32)
    w_proj = np.asarray(w_proj, np.float32)

    # RoPE tables in qkt partition order: partition r (within a 64-row head
    # block): quadrant qd = (r%64)//32, slot s = r%32, half = s//16 (0=even
    # feature, 1=odd), freq f = 16*qd + s%16.  Sign of sin is folded in:
    # even slots get -sin (r1 = x1 c - x2 s), odd slots +sin.
    r = np.arange(128)
    w = r % 64
    qd, s = w // 32, w % 32
    half, f = s // 16, 16 * qd + (s % 16)
    cct = cos.T[f, :]                                   # [128, T]
    sst = np.where((half == 0)[:, None], -sin.T[f, :], sin.T[f, :])
    # pre-permute sin rows by the shuffle partner p -> p^16 (per quadrant):
    # u[p] = q[p]*sst_pre[p], shuffle(u)[p] = q[p^16]*sst[p]
    partner = (r // 32) * 32 + ((r % 32) ^ 16)
    sst = sst[partner]
    tri01 = (np.arange(128)[None, :] >= np.arange(128)[:, None])  # [j, i] i>=j

    x_b = [np.ascontiguousarray(x[b].T).astype(BF) for b in range(B)]

    wq = w_qkv[:, 0:D]
    wk = w_qkv[:, D:2 * D] * np.float32(1.0 / np.sqrt(HD))
    wv = w_qkv[:, 2 * D:3 * D]

    def build_qk_aug(g):
        # per head: 64 columns ordered [qd][half][j] -> original col 2f+half
        cols = []
        for wm in (wq, wk):
            for pr in range(4):
                for hl in (0, 1):
                    hw = wm[:, (g * 8 + 2 * pr + hl) * 64:
                            (g * 8 + 2 * pr + hl + 1) * 64]
                    cols.append(hw[:, (2 * f + half)[:64]])
        return np.concatenate(cols, axis=1)             # [D, 1024]

    wqk_g = []
    for g in range(2):
        wa = build_qk_aug(g)                            # [1024(rows), 1024]
        wqk_g.append(np.ascontiguousarray(
            wa.reshape(NKT, 128, 1024).transpose(1, 0, 2)).astype(BF))
    wv_g = [np.ascontiguousarray(
        wv[:, g * 512:(g + 1) * 512].reshape(NKT, 128, 512)
        .transpose(1, 0, 2)).astype(BF) for g in range(2)]
    wp_g = [np.ascontiguousarray(
        w_proj[g * 512:(g + 1) * 512, :].reshape(4, 128, D)
        .transpose(1, 0, 2)).astype(BF) for g in range(2)]

    in_maps = []
    for c in range(N_CORES):
        b, g = c // 2, c % 2
        in_maps.append({
            "x": x_b[b], "wqk": wqk_g[g], "wv": wv_g[g], "wproj": wp_g[g],
            "cc": cct.astype(BF), "ss": sst.astype(BF),
            "tri": tri01.astype(BF),
        })
    return in_maps


def kernel(x, cos, sin, w_qkv, w_proj):
    nc = _get_program()
    in_maps = _host_inputs(x, cos, sin, w_qkv, w_proj)
    res = run_bass_kernel_spmd(nc, in_maps, core_ids=list(range(N_CORES)))
    out = np.empty((B, T, D), dtype=np.float32)
    for b in range(B):
        out[b] = res.results[2 * b]["out"] + res.results[2 * b + 1]["out"]
    return out



# revision 19
# speedup vs baseline: 1.2903x; 1.2903x over previous
"""Causal self-attention with RoPE on 8 Trainium2 NeuronCores.

Reference computation (B=4, T=2048, D=1024, H=16, hd=64, fp32):
    qkv = x @ w_qkv ; q,k per-head RoPE (interleaved pairs) ;
    out = softmax(causal(q k^T / 8)) @ v ; out @ w_proj

Sharding: core c -> (batch b = c//2, head-group g = c%2 of 8 heads).
Data parallel on B, tensor parallel on heads; w_proj is row-parallel so each
core returns a partial [2048, 1024] product (bf16) and the host sums the two
partials per batch in f32.

Per-core device program, all matmul inputs bf16 (fp32 PSUM accumulation),
engine-balanced so the PE (the roofline engine at ~228us of matmul work)
never idles:

  1. x^T, weights, RoPE tables prepared host-side in bf16.  The q/k weight
     columns are permuted per head into [quadrant][even|odd] order so the
     RoPE partner swap becomes a single DVE stream_shuffle (mask i^16 within
     each 32-partition quadrant); the rotation sign is folded into the sin
     table, the 1/sqrt(hd) score scale into the k columns.  wqk DRAM layout
     is [128, ft, kt, 128] so each per-ft weight DMA has 2KB contiguous
     runs (full 360GB/s bandwidth).
  2. Pair-0 q/k computed kt-outer into 8 PSUM accumulators so the PE chases
     the x DMA stream.  RoPE per chunk: one PSUM->bf16 copy on ACT/Pool,
     then 4 all-bf16-SBUF DVE ops (2 mults + shuffle + add) at the DVE 2x
     rate.
  3. Attention per (pair, i-half, head) is processed per 512-column segment:
     matmul into a 1-bank st PSUM tile, ACT exp -> bf16 P^T segment, causal
     diagonal via a bf16 DVE multiply with a 0/1 triangle, PV accumulated as
     matmul(lhsT=[v | ones], rhs=P^T) -> [65, 1024] PSUM (denominators fall
     out of the ones column).  Normalize: DVE reciprocal straight from the
     PSUM denominator row, Pool partition_broadcast, DVE multiply.
  4. The emission order is a single software-pipelined sequence
     A0,B0,A1,B1,A2,B2,A3,B3 (A = i-half 0, PE-heavy; B = i-half 1,
     ACT-heavy) with PE-only filler work (v tiles, later qkv pairs,
     projection tiles) woven into each stream at ratios sized to the local
     ACT deficit, so the in-order PE queue never head-of-line blocks on the
     exp chain.  PSUM: scratch(2 banks: v/qkv/proj) + st(2x1) + atps(2x2)
     = 8, allocated in clean LIFO order.
  5. out_partial = attn^T @ w_proj per token tile -> bf16 SBUF -> DRAM.
"""

import numpy as np
import ml_dtypes

import concourse.tile as tile
from concourse import bacc, mybir
from concourse.bass_utils import run_bass_kernel_spmd

F32 = mybir.dt.float32
BF16 = mybir.dt.bfloat16
AF = mybir.ActivationFunctionType
OP = mybir.AluOpType

B, T, D, NH, HD = 4, 2048, 1024, 16, 64
HPC = 8            # heads per core
N_CORES = 8
NTT = T // 128     # 16 token tiles
NKT = D // 128     # 8 contraction tiles
SHUF_MASK = [i ^ 16 for i in range(32)]

BF = ml_dtypes.bfloat16


def _build_program(reps: int = 1):
    nc = bacc.Bacc("TRN2", target_bir_lowering=False, debug=False)
    x_d = nc.dram_tensor("x", [D, T], BF16, kind="ExternalInput")     # x^T
    wqk_d = nc.dram_tensor("wqk", [128, 8, NKT, 128], BF16, kind="ExternalInput")
    wv_d = nc.dram_tensor("wv", [128, NKT, 512], BF16, kind="ExternalInput")
    wp_d = nc.dram_tensor("wproj", [128, 4, D], BF16, kind="ExternalInput")
    cc_d = nc.dram_tensor("cc", [128, T], BF16, kind="ExternalInput")
    ss_d = nc.dram_tensor("ss", [128, T], BF16, kind="ExternalInput")
    msk_d = nc.dram_tensor("msk", [128, 256], BF16, kind="ExternalInput")
    out_d = nc.dram_tensor("out", [T, D], BF16, kind="ExternalOutput")

    with tile.TileContext(nc) as tc:
      for _rep in range(reps):
        with (
            tc.tile_pool(name="persist", bufs=1) as pers,
            tc.tile_pool(name="vo", bufs=1) as vop,
            tc.tile_pool(name="qkt", bufs=1) as qktp,
            tc.tile_pool(name="at", bufs=1) as atp,
            tc.tile_pool(name="xt", bufs=1) as xtp,
            tc.tile_pool(name="ftw", bufs=4) as ftwp,
            tc.tile_pool(name="rope", bufs=3) as ropetp,
            tc.tile_pool(name="qraw", bufs=8) as qrawp,
            tc.tile_pool(name="pt", bufs=6) as ptp,
            tc.tile_pool(name="nrm", bufs=2) as nrmp,
            tc.tile_pool(name="osb", bufs=2) as osbp,
        ):
            msk = pers.tile([128, 256], BF16, tag="msk")
            cc = pers.tile([128, T], BF16, tag="cc")
            ss = pers.tile([128, T], BF16, tag="ss")
            wv_sb = pers.tile([128, NKT, 512], BF16, tag="wv")
            wp_sb = pers.tile([128, 4, D], BF16, tag="wp")
            xt = [xtp.tile([128, T], BF16, tag=f"xt{kt}", name=f"xt{kt}")
                  for kt in range(NKT)]

            qkt = [qktp.tile([128, T], BF16, tag=f"qkt{i}", name=f"qkt{i}")
                   for i in range(8)]
            vo = [vop.tile([128, HPC, 65], BF16, tag=f"vo{tt}", name=f"vo{tt}")
                  for tt in range(NTT)]
            at = [atp.tile([128, T], BF16, tag=f"at{pr}", name=f"at{pr}")
                  for pr in range(4)]

            # warm the ACT exp table before it's on the critical path
            warm = ropetp.tile([1, 8], F32, tag="warm")
            nc.vector.memset(warm[:], 0.0)
            nc.scalar.activation(warm[:], warm[:], AF.Exp)

            def emit_rope_copy(ps_a, ft, tcn, on_act):
                # qraw = bf16 copy of the PSUM q/k tile.  Only ACT and DVE
                # may read PSUM (GPSIMD/Pool cannot).
                qraw = qrawp.tile([128, 512], BF16, tag="qraw",
                                  name=f"qraw{ft}_{tcn}")
                if on_act:
                    nc.scalar.copy(qraw[:], ps_a[:])
                else:
                    nc.vector.tensor_copy(qraw[:], ps_a[:])
                return qraw

            def emit_rope_math(qraw, ft, tcn):
                # all-bf16-SBUF DVE ops run at the 2x rate.
                # u = q * ssn_pre (row-permuted sin table), so
                # shuffle(u)[p] = q[p^16] * ssn[p] == the RoPE partner term.
                sl = slice(tcn * 512, (tcn + 1) * 512)
                u = ropetp.tile([128, 512], BF16, tag="u",
                                name=f"u{ft}_{tcn}")
                t1 = ropetp.tile([128, 512], BF16, tag="t1")
                t2 = ropetp.tile([128, 512], BF16, tag="t2")
                nc.vector.tensor_tensor(u[:], qraw[:], ss[:, sl], OP.mult)
                nc.vector.tensor_tensor(t1[:], qraw[:], cc[:, sl], OP.mult)
                nc.vector.stream_shuffle(t2[:], u[:], SHUF_MASK)
                nc.vector.tensor_tensor(qkt[ft][:, sl], t1[:], t2[:], OP.add)

            def emit_rope(ps_a, ft, tcn, on_act):
                emit_rope_math(emit_rope_copy(ps_a, ft, tcn, on_act), ft, tcn)

            # ---- DMA priority order (one serial 360GB/s resource) ----
            wa0 = {}
            for side in range(2):
                ft = 4 * side
                wa0[ft] = ftwp.tile([128, NKT, 128], BF16, tag="wa",
                                    name=f"wa{ft}")
            nc.sync.dma_start(wa0[0][:, 0:1, :], wqk_d[:, 0, 0:1, :])
            nc.sync.dma_start(xt[0][:, 0:512], x_d[0:128, 0:512])
            nc.sync.dma_start(xt[0][:, 512:T], x_d[0:128, 512:T])
            nc.sync.dma_start(wa0[4][:, 0:1, :], wqk_d[:, 4, 0:1, :])
            nc.sync.dma_start(xt[1][:], x_d[128:256, :])
            nc.sync.dma_start(wa0[0][:, 1:NKT, :], wqk_d[:, 0, 1:NKT, :])
            nc.sync.dma_start(wa0[4][:, 1:NKT, :], wqk_d[:, 4, 1:NKT, :])
            for kt in range(2, NKT):
                nc.sync.dma_start(xt[kt][:], x_d[kt * 128:(kt + 1) * 128, :])
            nc.sync.dma_start(wv_sb[:], wv_d[:])
            nc.sync.dma_start(cc[:], cc_d[:])
            nc.sync.dma_start(ss[:], ss_d[:])
            nc.sync.dma_start(msk[:], msk_d[:])
            nc.sync.dma_start(wp_sb[:], wp_d[:])

            # ---- pair 0: kt-outer with 8 PSUM accumulators so the PE
            # chases the x DMA stream.
            with tc.tile_pool(name="qk0ps", bufs=1, space="PSUM") as qk0ps:
                acc = {}
                for side in range(2):
                    for tcn in range(4):
                        acc[(side, tcn)] = qk0ps.tile(
                            [128, 512], F32, tag=f"a{side}{tcn}",
                            name=f"qk0a{side}_{tcn}")
                for kt in range(NKT):
                    for side in range(2):
                        for tcn in range(4):
                            sl = slice(tcn * 512, (tcn + 1) * 512)
                            nc.tensor.matmul(
                                acc[(side, tcn)][:], wa0[4 * side][:, kt, :],
                                xt[kt][:, sl],
                                start=(kt == 0), stop=(kt == NKT - 1),
                            )
                # rope pair 0: all 8 PSUM->bf16 copies drain on ACT
                # right after the accumulators stop (qraw bufs=8), which
                # releases the qk0ps pool for the scratch pool below.
                qraws = {}
                for side in range(2):
                    for tcn in range(4):
                        qraws[(side, tcn)] = emit_rope_copy(
                            acc[(side, tcn)], 4 * side, tcn,
                            on_act=(tcn % 2 == 0))
                for side in range(2):
                    for tcn in range(4):
                        emit_rope_math(qraws[(side, tcn)], 4 * side, tcn)

            with tc.tile_pool(name="scratch", bufs=2, space="PSUM") as scr:
              with (
                tc.tile_pool(name="stps", bufs=2, space="PSUM") as stps,
                tc.tile_pool(name="atps", bufs=2, space="PSUM") as atps,
              ):
                # ---- QKV steps (pairs 1..3), v steps, proj steps --------
                def qk_steps(pr):
                    # emitted tcn-major (q tcn0, k tcn0, q tcn1, ...) so the
                    # first chunks cover the earliest attention reads and the
                    # tail chunks can be deferred as late fillers
                    loads, chunks = [], []
                    for side in range(2):          # 0 = q, 1 = k
                        ft = 4 * side + pr
                        w_a = ftwp.tile([128, NKT, 128], BF16, tag="wa",
                                        name=f"wa{ft}")
                        def load_w(ft=ft, w_a=w_a):
                            nc.sync.dma_start(w_a[:], wqk_d[:, ft])
                        loads.append(load_w)
                        for tcn in range(4):
                            def chunk(ft=ft, w_a=w_a, tcn=tcn, side=side):
                                sl = slice(tcn * 512, (tcn + 1) * 512)
                                ps_a = scr.tile([128, 512], F32, tag="ps",
                                                name=f"qk{ft}_{tcn}")
                                for kt in range(NKT):
                                    nc.tensor.matmul(
                                        ps_a[:], w_a[:, kt, :], xt[kt][:, sl],
                                        start=(kt == 0), stop=(kt == NKT - 1),
                                    )
                                emit_rope(ps_a, ft, tcn, on_act=False)
                            chunks.append((tcn, side, chunk))
                    chunks.sort(key=lambda c: (c[0], c[1]))
                    return loads + [c for _, _, c in chunks]

                def v_steps(tt_range):
                    steps = []
                    for tt in tt_range:
                        def vchunk(tt=tt):
                            nc.gpsimd.memset(vo[tt][:, :, 64:65], 1.0)
                            ps = scr.tile([128, 512], F32, tag="ps",
                                          name=f"v{tt}")
                            for kt in range(NKT):
                                nc.tensor.matmul(
                                    ps[:],
                                    xt[kt][:, tt * 128:(tt + 1) * 128],
                                    wv_sb[:, kt, :],
                                    start=(kt == 0), stop=(kt == NKT - 1),
                                )
                            nc.scalar.copy(
                                vo[tt][:, :, 0:64],
                                ps[:].rearrange("p (h d) -> p h d", h=HPC),
                            )
                        steps.append(vchunk)
                    return steps

                def proj_steps(tt_range, tail=False, pool=None):
                    pool = pool or scr
                    steps = []
                    for tt in tt_range:
                        def ptt(tt=tt):
                            o_sb = osbp.tile([128, D], BF16, tag="osb")
                            for nch in range(2):
                                pp = pool.tile([128, 512], F32, tag="ps",
                                               name=f"pp{tt}_{nch}")
                                for mt in range(4):
                                    nc.tensor.matmul(
                                        pp[:],
                                        at[mt][:, tt * 128:(tt + 1) * 128],
                                        wp_sb[:, mt,
                                              nch * 512:(nch + 1) * 512],
                                        start=(mt == 0), stop=(mt == 3),
                                    )
                                osl = slice(nch * 512, (nch + 1) * 512)
                                if not tail:
                                    nc.vector.tensor_copy(o_sb[:, osl], pp[:])
                                elif nch == 0:
                                    nc.scalar.copy(o_sb[:, osl], pp[:])
                                else:
                                    nc.vector.tensor_copy(o_sb[:, osl], pp[:])
                                if tail:
                                    nc.sync.dma_start(
                                        out_d[tt * 128:(tt + 1) * 128, osl],
                                        o_sb[:, osl])
                            if not tail:
                                nc.sync.dma_start(
                                    out_d[tt * 128:(tt + 1) * 128, :], o_sb[:])
                        steps.append(ptt)
                    return steps

                def attn_q(pr, ih, iq):
                    """Per (pair, i-half): i-block-major, both heads packed
                    into one st/pt tile so each exp call covers two heads.

                    For each 512-column i-block L: loop jt ascending over
                    the contributing key tiles; per (jt, block) segment
                    [lo, hi) = [max(L, j0), L+512):
                      st[:, 0:w]   = scores head0, st[:, w:2w] = head1
                      one exp [128, 2w], tri on each head's diag sub-block,
                      PV per head into its [65, 512] accumulator.
                    Accumulators complete per block -> normalize per head.
                    """
                    i0 = 1024 * ih
                    steps = []

                    if True:
                        L = i0 + 512 * iq
                        n_jt = (L + 512) // 128  # jts with j0 < L+512
                        segs = []
                        for jt in range(n_jt):
                            j0 = 128 * jt
                            segs.append((jt, max(L, j0), L + 512))

                        at_ps = {}

                        def get_at_ps(h, at_ps=at_ps, ih=ih, iq=iq):
                            if h not in at_ps:
                                at_ps[h] = atps.tile(
                                    [65, 512], F32, tag="atps",
                                    name=f"at{ih}_{iq}_{h}")
                            return at_ps[h]

                        def emit_st(seg, ih=ih, iq=iq):
                            jt, lo, hi = seg
                            j0 = 128 * jt
                            w = hi - lo
                            diag = lo == j0
                            # head-1 offset: a matmul output may not cross a
                            # 2KB PSUM bank boundary, so when the pair does
                            # not fit in bank 0 put head 1 at column 512
                            # (bank 1).  The exp then also covers the
                            # [w, 512) gap; st only ever holds scores, so
                            # exp of stale values is finite and unread.
                            o1 = 2 * w if 2 * w <= 512 else 512 + w
                            h1o = o1 - w
                            st = stps.tile([128, 1024], F32, tag="st",
                                           name=f"st{ih}_{iq}_{jt}")
                            # PSUM accumulation groups are bank-granular:
                            # close head 0's group (score then mask) before
                            # opening head 1's when both share bank 0.
                            for hx in range(2):
                                r0 = 64 * hx
                                d0 = hx * h1o
                                nc.tensor.matmul(
                                    st[:, d0:d0 + w],
                                    qkt[4 + pr][r0:r0 + 64, j0:j0 + 128],
                                    qkt[pr][r0:r0 + 64, lo:hi],
                                    start=True, stop=not diag,
                                )
                                if diag:
                                    # causal mask folded into the scores:
                                    # add -60000 * [j > i] to the diagonal
                                    # block via a tiny PE matmul
                                    # (negI @ triU), so exp yields exact
                                    # zeros with no vector-engine op on pt.
                                    nc.tensor.matmul(
                                        st[:, d0:d0 + 128],
                                        msk[:, 128:256], msk[:, 0:128],
                                        start=False, stop=True,
                                    )
                                if diag and hx == 0 and w == 384:
                                    # define the [w, 512) bank-alignment gap
                                    # so the combined exp reads no stale
                                    # PSUM (values are unread by the PV)
                                    nc.tensor.matmul(
                                        st[:, w:512],
                                        msk[:, 128:256], msk[:, 0:128],
                                        start=True, stop=True,
                                    )
                            pt = ptp.tile([128, 1024], BF16, tag="pt",
                                          name=f"pt{ih}_{iq}_{jt}")
                            nc.scalar.activation(
                                pt[:, 0:o1], st[:, 0:o1], AF.Exp)
                            return pt, h1o

                        def emit_pv(seg, pth, get_at_ps=get_at_ps,
                                    n_jt=n_jt, L=L):
                            jt, lo, hi = seg
                            pt, h1o = pth
                            w = hi - lo
                            last_jt = n_jt - 1
                            for hx in range(2):
                                d0 = hx * h1o
                                nc.tensor.matmul(
                                    get_at_ps(hx)[:, lo - L:hi - L],
                                    vo[jt][:, 2 * pr + hx, :],
                                    pt[:, d0:d0 + w],
                                    start=(jt == 0), stop=(jt == last_jt),
                                )

                        def normalize(get_at_ps=get_at_ps, L=L):
                            for hx in range(2):
                                at_ps = get_at_ps(hx)
                                den = nrmp.tile([1, 512], F32, tag="den")
                                r_sb = nrmp.tile([1, 512], F32, tag="r")
                                rb_sb = nrmp.tile([64, 512], F32, tag="rb")
                                # custom DVE ops mis-address PSUM at nonzero
                                # partition offsets on real HW: stage the
                                # denominator row through SBUF first
                                nc.vector.tensor_copy(
                                    den[:], at_ps[64:65, :])
                                nc.vector.reciprocal_approx_fast(
                                    r_sb[:], den[:])
                                nc.gpsimd.partition_broadcast(
                                    rb_sb[:], r_sb[:])
                                nc.vector.tensor_tensor(
                                    at[pr][64 * hx:64 * hx + 64, L:L + 512],
                                    at_ps[0:64, :], rb_sb[:], OP.mult)

                        def step(k, segs=segs, emit_st=emit_st,
                                 emit_pv=emit_pv, normalize=normalize,
                                 state={"prev": None}):
                            pt = emit_st(segs[k])
                            if state["prev"] is not None:
                                emit_pv(segs[k - 1], state["prev"])
                            state["prev"] = pt
                            if k == len(segs) - 1:
                                emit_pv(segs[k], pt)
                                normalize()

                        for k in range(len(segs)):
                            steps.append(lambda k=k, step=step: step(k))
                    return steps

                def merge(a_steps, b_steps):
                    out = []
                    na, nb = len(a_steps), len(b_steps)
                    j = 0
                    for i, s in enumerate(a_steps):
                        out.append(s)
                        while j * na < (i + 1) * nb:
                            out.append(b_steps[j])
                            j += 1
                    out.extend(b_steps[j:])
                    return out

                # ---- the big software-pipelined sequence ---------------
                # Quarters Xp_q = attn(pair p, i-half X, 512-block q), with
                # fillers (v tiles, later qkv pairs, proj tiles) sized to
                # each quarter's ACT deficit.  v_k must be emitted before
                # any pv that reads vo[k] (in-order PE queue).
                qk1 = qk_steps(1)
                qk2 = qk_steps(2)
                qk3 = qk_steps(3)
                rows = [
                    (v_steps(range(0, 6)), []),          # lead-in
                    (attn_q(0, 0, 0), v_steps(range(6, 11))),
                    (attn_q(0, 0, 1), v_steps(range(11, 16))),
                    (attn_q(0, 1, 0), qk1[:4]),   # loads + q/k tcn0
                    (attn_q(0, 1, 1), qk1[4:8]),  # tcn1, tcn2
                    (attn_q(1, 0, 0), qk1[8:] + qk2[:2]),
                    (attn_q(1, 0, 1), qk2[2:4]),  # qk2 tcn0
                    (attn_q(1, 1, 0), qk2[4:6]),
                    (attn_q(1, 1, 1), qk2[6:8]),
                    (attn_q(2, 0, 0), qk2[8:] + qk3[:2]),
                    (attn_q(2, 0, 1), qk3[2:4]),  # qk3 tcn0
                    (attn_q(2, 1, 0), qk3[4:6]),
                    (attn_q(2, 1, 1), qk3[6:8]),
                    (attn_q(3, 0, 0), qk3[8:]),   # qk3 tcn3 (for B3q1)
                    (attn_q(3, 0, 1), proj_steps(range(0, 3))),
                    (attn_q(3, 1, 0), proj_steps(range(3, 8))),
                    (attn_q(3, 1, 1), proj_steps(range(8, 12))),
                    (proj_steps(range(12, 16), tail=True), []),
                ]
                for a, b in rows:
                    for s in merge(a, b):
                        s()
    nc.compile()
    return nc


_NC_CACHE = None


def _get_program():
    global _NC_CACHE
    if _NC_CACHE is None:
        _NC_CACHE = _build_program()
    return _NC_CACHE


def _host_inputs(x, cos, sin, w_qkv, w_proj):
    """Build the 8 per-core input dicts (bf16, pre-permuted layouts)."""
    x = np.asarray(x, np.float32)
    cos = np.asarray(cos, np.float32)
    sin = np.asarray(sin, np.float32)
    w_qkv = np.asarray(w_qkv, np.float32)
    w_proj = np.asarray(w_proj, np.float32)

    # RoPE tables in qkt partition order: partition r (within a 64-row head
    # block): quadrant qd = (r%64)//32, slot s = r%32, half = s//16 (0=even
    # feature, 1=odd), freq f = 16*qd + s%16.  Sign of sin is folded in:
    # even slots get -sin (r1 = x1 c - x2 s), odd slots +sin.
    r = np.arange(128)
    w = r % 64
    qd, s = w // 32, w % 32
    half, f = s // 16, 16 * qd + (s % 16)
    cct = cos.T[f, :]                                   # [128, T]
    sst = np.where((half == 0)[:, None], -sin.T[f, :], sin.T[f, :])
    # pre-permute sin rows by the shuffle partner p -> p^16 (per quadrant):
    # u[p] = q[p]*sst_pre[p], shuffle(u)[p] = q[p^16]*sst[p]
    partner = (r // 32) * 32 + ((r % 32) ^ 16)
    sst = sst[partner]
    # mask pair for the diagonal-block PE matmul: st += negI^T @ triU
    triu = (np.arange(128)[None, :] < np.arange(128)[:, None])   # [j, i] j>i
    negi = np.eye(128, dtype=np.float32) * np.float32(-60000.0)
    msk = np.concatenate([triu.astype(np.float32), negi], axis=1)

    x_b = [np.ascontiguousarray(x[b].T).astype(BF) for b in range(B)]

    wq = w_qkv[:, 0:D]
    wk = w_qkv[:, D:2 * D] * np.float32(1.0 / np.sqrt(HD))
    wv = w_qkv[:, 2 * D:3 * D]

    def build_qk_aug(g):
        # per head: 64 columns ordered [qd][half][j] -> original col 2f+half
        cols = []
        for wm in (wq, wk):
            for pr in range(4):
                for hl in (0, 1):
                    hw = wm[:, (g * 8 + 2 * pr + hl) * 64:
                            (g * 8 + 2 * pr + hl + 1) * 64]
                    cols.append(hw[:, (2 * f + half)[:64]])
        return np.concatenate(cols, axis=1)             # [D, 1024]

    wqk_g = []
    for g in range(2):
        wa = build_qk_aug(g)                            # [1024(rows), 1024]
        # device layout [128, ft, kt, 128]: wqk[p, ft, kt, c] =
        # wa[kt*128+p, ft*128+c] -> per-ft slices are contiguous 2KB runs
        wqk_g.append(np.ascontiguousarray(
            wa.reshape(NKT, 128, 8, 128).transpose(1, 2, 0, 3)).astype(BF))
    wv_g = [np.ascontiguousarray(
        wv[:, g * 512:(g + 1) * 512].reshape(NKT, 128, 512)
        .transpose(1, 0, 2)).astype(BF) for g in range(2)]
    wp_g = [np.ascontiguousarray(
        w_proj[g * 512:(g + 1) * 512, :].reshape(4, 128, D)
        .transpose(1, 0, 2)).astype(BF) for g in range(2)]

    in_maps = []
    for c in range(N_CORES):
        b, g = c // 2, c % 2
        in_maps.append({
            "x": x_b[b], "wqk": wqk_g[g], "wv": wv_g[g], "wproj": wp_g[g],
            "cc": cct.astype(BF), "ss": sst.astype(BF),
            "msk": msk.astype(BF),
        })
    return in_maps


def kernel(x, cos, sin, w_qkv, w_proj):
    nc = _get_program()
    in_maps = _host_inputs(x, cos, sin, w_qkv, w_proj)
    res = run_bass_kernel_spmd(nc, in_maps, core_ids=list(range(N_CORES)))
    out = np.empty((B, T, D), dtype=np.float32)
    for b in range(B):
        out[b] = (res.results[2 * b]["out"].astype(np.float32)
                  + res.results[2 * b + 1]["out"].astype(np.float32))
    return out


# revision 23
# speedup vs baseline: 1.2910x; 1.0005x over previous
"""Causal self-attention with RoPE on 8 Trainium2 NeuronCores.

Reference computation (B=4, T=2048, D=1024, H=16, hd=64, fp32):
    qkv = x @ w_qkv ; q,k per-head RoPE (interleaved pairs) ;
    out = softmax(causal(q k^T / 8)) @ v ; out @ w_proj

Sharding: core c -> (batch b = c//2, head-group g = c%2 of 8 heads).
Data parallel on B, tensor parallel on heads; w_proj is row-parallel so each
core returns a partial [2048, 1024] product (bf16) and the host sums the two
partials per batch in f32.

Per-core device program, all matmul inputs bf16 (fp32 PSUM accumulation),
engine-balanced so the PE (the roofline engine at ~228us of matmul work)
never idles:

  1. x^T, weights, RoPE tables prepared host-side in bf16.  The q/k weight
     columns are permuted per head into [quadrant][even|odd] order so the
     RoPE partner swap becomes a single DVE stream_shuffle (mask i^16 within
     each 32-partition quadrant); the rotation sign is folded into the sin
     table, the 1/sqrt(hd) score scale into the k columns.  wqk DRAM layout
     is [128, ft, kt, 128] so each per-ft weight DMA has 2KB contiguous
     runs (full 360GB/s bandwidth).
  2. Pair-0 q/k computed kt-outer into 8 PSUM accumulators so the PE chases
     the x DMA stream.  RoPE per chunk: one PSUM->bf16 copy on ACT/Pool,
     then 4 all-bf16-SBUF DVE ops (2 mults + shuffle + add) at the DVE 2x
     rate.
  3. Attention per (pair, i-half, head) is processed per 512-column segment:
     matmul into a 1-bank st PSUM tile, ACT exp -> bf16 P^T segment, causal
     diagonal via a bf16 DVE multiply with a 0/1 triangle, PV accumulated as
     matmul(lhsT=[v | ones], rhs=P^T) -> [65, 1024] PSUM (denominators fall
     out of the ones column).  Normalize: DVE reciprocal straight from the
     PSUM denominator row, Pool partition_broadcast, DVE multiply.
  4. The emission order is a single software-pipelined sequence
     A0,B0,A1,B1,A2,B2,A3,B3 (A = i-half 0, PE-heavy; B = i-half 1,
     ACT-heavy) with PE-only filler work (v tiles, later qkv pairs,
     projection tiles) woven into each stream at ratios sized to the local
     ACT deficit, so the in-order PE queue never head-of-line blocks on the
     exp chain.  PSUM: scratch(2 banks: v/qkv/proj) + st(2x1) + atps(2x2)
     = 8, allocated in clean LIFO order.
  5. out_partial = attn^T @ w_proj per token tile -> bf16 SBUF -> DRAM.
"""

import numpy as np
import ml_dtypes

import concourse.tile as tile
from concourse import bacc, mybir
from concourse.bass_utils import run_bass_kernel_spmd

F32 = mybir.dt.float32
BF16 = mybir.dt.bfloat16
AF = mybir.ActivationFunctionType
OP = mybir.AluOpType

B, T, D, NH, HD = 4, 2048, 1024, 16, 64
HPC = 8            # heads per core
N_CORES = 8
NTT = T // 128     # 16 token tiles
NKT = D // 128     # 8 contraction tiles
SHUF_MASK = [i ^ 16 for i in range(32)]

BF = ml_dtypes.bfloat16


def _build_program(reps: int = 1):
    nc = bacc.Bacc("TRN2", target_bir_lowering=False, debug=False)
    x_d = nc.dram_tensor("x", [D, T], BF16, kind="ExternalInput")     # x^T
    wqk_d = nc.dram_tensor("wqk", [128, 8, NKT, 128], BF16, kind="ExternalInput")
    wv_d = nc.dram_tensor("wv", [128, NKT, 512], BF16, kind="ExternalInput")
    wp_d = nc.dram_tensor("wproj", [128, 4, D], BF16, kind="ExternalInput")
    cc_d = nc.dram_tensor("cc", [128, T], BF16, kind="ExternalInput")
    ss_d = nc.dram_tensor("ss", [128, T], BF16, kind="ExternalInput")
    msk_d = nc.dram_tensor("msk", [128, 256], BF16, kind="ExternalInput")
    out_d = nc.dram_tensor("out", [T, D], BF16, kind="ExternalOutput")

    with tile.TileContext(nc) as tc:
      for _rep in range(reps):
        with (
            tc.tile_pool(name="persist", bufs=1) as pers,
            tc.tile_pool(name="vo", bufs=1) as vop,
            tc.tile_pool(name="qkt", bufs=1) as qktp,
            tc.tile_pool(name="at", bufs=1) as atp,
            tc.tile_pool(name="xt", bufs=1) as xtp,
            tc.tile_pool(name="ftw", bufs=4) as ftwp,
            tc.tile_pool(name="rope", bufs=3) as ropetp,
            tc.tile_pool(name="qraw", bufs=8) as qrawp,
            tc.tile_pool(name="pt", bufs=8) as ptp,
            tc.tile_pool(name="nrm", bufs=3) as nrmp,
            tc.tile_pool(name="osb", bufs=3) as osbp,
        ):
            msk = pers.tile([128, 256], BF16, tag="msk")
            cc = pers.tile([128, T], BF16, tag="cc")
            ss = pers.tile([128, T], BF16, tag="ss")
            wv_sb = pers.tile([128, NKT, 512], BF16, tag="wv")
            wp_sb = pers.tile([128, 4, D], BF16, tag="wp")
            xt = [xtp.tile([128, T], BF16, tag=f"xt{kt}", name=f"xt{kt}")
                  for kt in range(NKT)]

            qkt = [qktp.tile([128, T], BF16, tag=f"qkt{i}", name=f"qkt{i}")
                   for i in range(8)]
            vo = [vop.tile([128, HPC, 65], BF16, tag=f"vo{tt}", name=f"vo{tt}")
                  for tt in range(NTT)]
            at = [atp.tile([128, T], BF16, tag=f"at{pr}", name=f"at{pr}")
                  for pr in range(4)]

            # warm the ACT exp table before it's on the critical path
            warm = ropetp.tile([1, 8], F32, tag="warm")
            nc.vector.memset(warm[:], 0.0)
            nc.scalar.activation(warm[:], warm[:], AF.Exp)

            def emit_rope_copy(ps_a, ft, tcn, on_act):
                # qraw = bf16 copy of the PSUM q/k tile.  Only ACT and DVE
                # may read PSUM (GPSIMD/Pool cannot).
                qraw = qrawp.tile([128, 512], BF16, tag="qraw",
                                  name=f"qraw{ft}_{tcn}")
                if on_act:
                    nc.scalar.copy(qraw[:], ps_a[:])
                else:
                    nc.vector.tensor_copy(qraw[:], ps_a[:])
                return qraw

            def emit_rope_math(qraw, ft, tcn):
                # all-bf16-SBUF DVE ops run at the 2x rate.
                # u = q * ssn_pre (row-permuted sin table), so
                # shuffle(u)[p] = q[p^16] * ssn[p] == the RoPE partner term.
                sl = slice(tcn * 512, (tcn + 1) * 512)
                u = ropetp.tile([128, 512], BF16, tag="u",
                                name=f"u{ft}_{tcn}")
                t1 = ropetp.tile([128, 512], BF16, tag="t1")
                t2 = ropetp.tile([128, 512], BF16, tag="t2")
                nc.vector.tensor_tensor(u[:], qraw[:], ss[:, sl], OP.mult)
                nc.vector.tensor_tensor(t1[:], qraw[:], cc[:, sl], OP.mult)
                nc.vector.stream_shuffle(t2[:], u[:], SHUF_MASK)
                nc.vector.tensor_tensor(qkt[ft][:, sl], t1[:], t2[:], OP.add)

            def emit_rope(ps_a, ft, tcn, on_act):
                emit_rope_math(emit_rope_copy(ps_a, ft, tcn, on_act), ft, tcn)

            # ---- DMA priority order (one serial 360GB/s resource) ----
            wa0 = {}
            for side in range(2):
                ft = 4 * side
                wa0[ft] = ftwp.tile([128, NKT, 128], BF16, tag="wa",
                                    name=f"wa{ft}")
            nc.sync.dma_start(wa0[0][:, 0:1, :], wqk_d[:, 0, 0:1, :])
            nc.sync.dma_start(xt[0][:, 0:512], x_d[0:128, 0:512])
            nc.sync.dma_start(xt[0][:, 512:T], x_d[0:128, 512:T])
            nc.sync.dma_start(wa0[4][:, 0:1, :], wqk_d[:, 4, 0:1, :])
            nc.sync.dma_start(xt[1][:], x_d[128:256, :])
            nc.sync.dma_start(wa0[0][:, 1:NKT, :], wqk_d[:, 0, 1:NKT, :])
            nc.sync.dma_start(wa0[4][:, 1:NKT, :], wqk_d[:, 4, 1:NKT, :])
            for kt in range(2, NKT):
                nc.sync.dma_start(xt[kt][:], x_d[kt * 128:(kt + 1) * 128, :])
            nc.sync.dma_start(wv_sb[:], wv_d[:])
            nc.sync.dma_start(cc[:], cc_d[:])
            nc.sync.dma_start(ss[:], ss_d[:])
            nc.sync.dma_start(msk[:], msk_d[:])
            nc.sync.dma_start(wp_sb[:], wp_d[:])

            # ---- pair 0: kt-outer with 8 PSUM accumulators so the PE
            # chases the x DMA stream.
            with tc.tile_pool(name="qk0ps", bufs=1, space="PSUM") as qk0ps:
                acc = {}
                for side in range(2):
                    for tcn in range(4):
                        acc[(side, tcn)] = qk0ps.tile(
                            [128, 512], F32, tag=f"a{side}{tcn}",
                            name=f"qk0a{side}_{tcn}")
                for kt in range(NKT):
                    for side in range(2):
                        for tcn in range(4):
                            sl = slice(tcn * 512, (tcn + 1) * 512)
                            nc.tensor.matmul(
                                acc[(side, tcn)][:], wa0[4 * side][:, kt, :],
                                xt[kt][:, sl],
                                start=(kt == 0), stop=(kt == NKT - 1),
                            )
                # rope pair 0: all 8 PSUM->bf16 copies drain on ACT
                # right after the accumulators stop (qraw bufs=8), which
                # releases the qk0ps pool for the scratch pool below.
                qraws = {}
                for side in range(2):
                    for tcn in range(4):
                        qraws[(side, tcn)] = emit_rope_copy(
                            acc[(side, tcn)], 4 * side, tcn,
                            on_act=(tcn % 2 == 0))
                for side in range(2):
                    for tcn in range(4):
                        emit_rope_math(qraws[(side, tcn)], 4 * side, tcn)

            with tc.tile_pool(name="scratch", bufs=2, space="PSUM") as scr:
              with (
                tc.tile_pool(name="stps", bufs=2, space="PSUM") as stps,
                tc.tile_pool(name="atps", bufs=2, space="PSUM") as atps,
              ):
                # ---- QKV steps (pairs 1..3), v steps, proj steps --------
                def qk_steps(pr):
                    # emitted tcn-major (q tcn0, k tcn0, q tcn1, ...) so the
                    # first chunks cover the earliest attention reads and the
                    # tail chunks can be deferred as late fillers
                    loads, chunks = [], []
                    for side in range(2):          # 0 = q, 1 = k
                        ft = 4 * side + pr
                        w_a = ftwp.tile([128, NKT, 128], BF16, tag="wa",
                                        name=f"wa{ft}")
                        def load_w(ft=ft, w_a=w_a):
                            nc.sync.dma_start(w_a[:], wqk_d[:, ft])
                        loads.append(load_w)
                        for tcn in range(4):
                            def chunk(ft=ft, w_a=w_a, tcn=tcn, side=side):
                                sl = slice(tcn * 512, (tcn + 1) * 512)
                                ps_a = scr.tile([128, 512], F32, tag="ps",
                                                name=f"qk{ft}_{tcn}")
                                for kt in range(NKT):
                                    nc.tensor.matmul(
                                        ps_a[:], w_a[:, kt, :], xt[kt][:, sl],
                                        start=(kt == 0), stop=(kt == NKT - 1),
                                    )
                                emit_rope(ps_a, ft, tcn, on_act=False)
                            chunks.append((tcn, side, chunk))
                    chunks.sort(key=lambda c: (c[0], c[1]))
                    return loads + [c for _, _, c in chunks]

                def v_steps(tt_range):
                    steps = []
                    for tt in tt_range:
                        def vchunk(tt=tt):
                            nc.gpsimd.memset(vo[tt][:, :, 64:65], 1.0)
                            ps = scr.tile([128, 512], F32, tag="ps",
                                          name=f"v{tt}")
                            for kt in range(NKT):
                                nc.tensor.matmul(
                                    ps[:],
                                    xt[kt][:, tt * 128:(tt + 1) * 128],
                                    wv_sb[:, kt, :],
                                    start=(kt == 0), stop=(kt == NKT - 1),
                                )
                            nc.scalar.copy(
                                vo[tt][:, :, 0:64],
                                ps[:].rearrange("p (h d) -> p h d", h=HPC),
                            )
                        steps.append(vchunk)
                    return steps

                def proj_steps(tt_range, tail=False, pool=None):
                    pool = pool or scr
                    steps = []
                    for tt in tt_range:
                        def ptt(tt=tt):
                            o_sb = osbp.tile([128, D], BF16, tag="osb")
                            for nch in range(2):
                                pp = pool.tile([128, 512], F32, tag="ps",
                                               name=f"pp{tt}_{nch}")
                                for mt in range(4):
                                    nc.tensor.matmul(
                                        pp[:],
                                        at[mt][:, tt * 128:(tt + 1) * 128],
                                        wp_sb[:, mt,
                                              nch * 512:(nch + 1) * 512],
                                        start=(mt == 0), stop=(mt == 3),
                                    )
                                osl = slice(nch * 512, (nch + 1) * 512)
                                if not tail:
                                    nc.vector.tensor_copy(o_sb[:, osl], pp[:])
                                elif nch == 0:
                                    nc.scalar.copy(o_sb[:, osl], pp[:])
                                else:
                                    nc.vector.tensor_copy(o_sb[:, osl], pp[:])
                                if tail:
                                    nc.sync.dma_start(
                                        out_d[tt * 128:(tt + 1) * 128, osl],
                                        o_sb[:, osl])
                            if not tail:
                                nc.sync.dma_start(
                                    out_d[tt * 128:(tt + 1) * 128, :], o_sb[:])
                        steps.append(ptt)
                    return steps

                def attn_q(pr, ih, iq):
                    """Per (pair, i-half): i-block-major, both heads packed
                    into one st/pt tile so each exp call covers two heads.

                    For each 512-column i-block L: loop jt ascending over
                    the contributing key tiles; per (jt, block) segment
                    [lo, hi) = [max(L, j0), L+512):
                      st[:, 0:w]   = scores head0, st[:, w:2w] = head1
                      one exp [128, 2w], tri on each head's diag sub-block,
                      PV per head into its [65, 512] accumulator.
                    Accumulators complete per block -> normalize per head.
                    """
                    i0 = 1024 * ih
                    steps = []

                    if True:
                        L = i0 + 512 * iq
                        n_jt = (L + 512) // 128  # jts with j0 < L+512
                        segs = []
                        for jt in range(n_jt):
                            j0 = 128 * jt
                            segs.append((jt, max(L, j0), L + 512))

                        at_ps = {}

                        def get_at_ps(h, at_ps=at_ps, ih=ih, iq=iq):
                            if h not in at_ps:
                                at_ps[h] = atps.tile(
                                    [65, 512], F32, tag="atps",
                                    name=f"at{ih}_{iq}_{h}")
                            return at_ps[h]

                        def emit_st(seg, ih=ih, iq=iq):
                            jt, lo, hi = seg
                            j0 = 128 * jt
                            w = hi - lo
                            diag = lo == j0
                            # head-1 offset: a matmul output may not cross a
                            # 2KB PSUM bank boundary, so when the pair does
                            # not fit in bank 0 put head 1 at column 512
                            # (bank 1).  The exp then also covers the
                            # [w, 512) gap; st only ever holds scores, so
                            # exp of stale values is finite and unread.
                            o1 = 2 * w if 2 * w <= 512 else 512 + w
                            h1o = o1 - w
                            st = stps.tile([128, 1024], F32, tag="st",
                                           name=f"st{ih}_{iq}_{jt}")
                            # PSUM accumulation groups are bank-granular:
                            # close head 0's group (score then mask) before
                            # opening head 1's when both share bank 0.
                            for hx in range(2):
                                r0 = 64 * hx
                                d0 = hx * h1o
                                nc.tensor.matmul(
                                    st[:, d0:d0 + w],
                                    qkt[4 + pr][r0:r0 + 64, j0:j0 + 128],
                                    qkt[pr][r0:r0 + 64, lo:hi],
                                    start=True, stop=not diag,
                                )
                                if diag:
                                    # causal mask folded into the scores:
                                    # add -60000 * [j > i] to the diagonal
                                    # block via a tiny PE matmul
                                    # (negI @ triU), so exp yields exact
                                    # zeros with no vector-engine op on pt.
                                    nc.tensor.matmul(
                                        st[:, d0:d0 + 128],
                                        msk[:, 128:256], msk[:, 0:128],
                                        start=False, stop=True,
                                    )
                                if diag and hx == 0 and w == 384:
                                    # define the [w, 512) bank-alignment gap
                                    # so the combined exp reads no stale
                                    # PSUM (values are unread by the PV)
                                    nc.tensor.matmul(
                                        st[:, w:512],
                                        msk[:, 128:256], msk[:, 0:128],
                                        start=True, stop=True,
                                    )
                            pt = ptp.tile([128, 1024], BF16, tag="pt",
                                          name=f"pt{ih}_{iq}_{jt}")
                            nc.scalar.activation(
                                pt[:, 0:o1], st[:, 0:o1], AF.Exp)
                            return pt, h1o

                        def emit_pv(seg, pth, get_at_ps=get_at_ps,
                                    n_jt=n_jt, L=L):
                            jt, lo, hi = seg
                            pt, h1o = pth
                            w = hi - lo
                            last_jt = n_jt - 1
                            for hx in range(2):
                                d0 = hx * h1o
                                nc.tensor.matmul(
                                    get_at_ps(hx)[:, lo - L:hi - L],
                                    vo[jt][:, 2 * pr + hx, :],
                                    pt[:, d0:d0 + w],
                                    start=(jt == 0), stop=(jt == last_jt),
                                )

                        def normalize(get_at_ps=get_at_ps, L=L):
                            for hx in range(2):
                                at_ps = get_at_ps(hx)
                                den = nrmp.tile([1, 512], F32, tag="den")
                                r_sb = nrmp.tile([1, 512], F32, tag="r")
                                rb_sb = nrmp.tile([64, 512], F32, tag="rb")
                                # custom DVE ops mis-address PSUM at nonzero
                                # partition offsets on real HW: stage the
                                # denominator row through SBUF first
                                nc.vector.tensor_copy(
                                    den[:], at_ps[64:65, :])
                                nc.vector.reciprocal_approx_fast(
                                    r_sb[:], den[:])
                                nc.gpsimd.partition_broadcast(
                                    rb_sb[:], r_sb[:])
                                nc.vector.tensor_tensor(
                                    at[pr][64 * hx:64 * hx + 64, L:L + 512],
                                    at_ps[0:64, :], rb_sb[:], OP.mult)

                        def step(k, segs=segs, emit_st=emit_st,
                                 emit_pv=emit_pv, normalize=normalize,
                                 state={"prev": None}):
                            pt = emit_st(segs[k])
                            if state["prev"] is not None:
                                emit_pv(segs[k - 1], state["prev"])
                            state["prev"] = pt
                            if k == len(segs) - 1:
                                emit_pv(segs[k], pt)
                                normalize()

                        for k in range(len(segs)):
                            steps.append(lambda k=k, step=step: step(k))
                    return steps

                def merge(a_steps, b_steps):
                    out = []
                    na, nb = len(a_steps), len(b_steps)
                    j = 0
                    for i, s in enumerate(a_steps):
                        out.append(s)
                        while j * na < (i + 1) * nb:
                            out.append(b_steps[j])
                            j += 1
                    out.extend(b_steps[j:])
                    return out

                # ---- the big software-pipelined sequence ---------------
                # Quarters Xp_q = attn(pair p, i-half X, 512-block q), with
                # fillers (v tiles, later qkv pairs, proj tiles) sized to
                # each quarter's ACT deficit.  v_k must be emitted before
                # any pv that reads vo[k] (in-order PE queue).
                qk1 = qk_steps(1)
                qk2 = qk_steps(2)
                qk3 = qk_steps(3)
                rows = [
                    (v_steps(range(0, 6)), []),          # lead-in
                    (attn_q(0, 0, 0), v_steps(range(6, 11))),
                    (attn_q(0, 0, 1), v_steps(range(11, 16))),
                    (attn_q(0, 1, 0), qk1[:4]),   # loads + q/k tcn0
                    (attn_q(0, 1, 1), qk1[4:8]),  # tcn1, tcn2
                    (attn_q(1, 0, 0), qk1[8:] + qk2[:2]),
                    (attn_q(1, 0, 1), qk2[2:4]),  # qk2 tcn0
                    (attn_q(1, 1, 0), qk2[4:6]),
                    (attn_q(1, 1, 1), qk2[6:8]),
                    (attn_q(2, 0, 0), qk2[8:] + qk3[:2]),
                    (attn_q(2, 0, 1), qk3[2:4]),  # qk3 tcn0
                    (attn_q(2, 1, 0), qk3[4:6]),
                    (attn_q(2, 1, 1), qk3[6:8]),
                    (attn_q(3, 0, 0), qk3[8:]),   # qk3 tcn3 (for B3q1)
                    (attn_q(3, 0, 1), proj_steps(range(0, 3))),
                    (attn_q(3, 1, 0), proj_steps(range(3, 8))),
                    (attn_q(3, 1, 1), proj_steps(range(8, 12))),
                    (proj_steps(range(12, 16), tail=True), []),
                ]
                for a, b in rows:
                    for s in merge(a, b):
                        s()
    nc.compile()
    return nc


_NC_CACHE = None


def _get_program():
    global _NC_CACHE
    if _NC_CACHE is None:
        _NC_CACHE = _build_program()
    return _NC_CACHE


def _host_inputs(x, cos, sin, w_qkv, w_proj):
    """Build the 8 per-core input dicts (bf16, pre-permuted layouts)."""
    x = np.asarray(x, np.float32)
    cos = np.asarray(cos, np.float32)
    sin = np.asarray(sin, np.float32)
    w_qkv = np.asarray(w_qkv, np.float32)
    w_proj = np.asarray(w_proj, np.float32)

    # RoPE tables in qkt partition order: partition r (within a 64-row head
    # block): quadrant qd = (r%64)//32, slot s = r%32, half = s//16 (0=even
    # feature, 1=odd), freq f = 16*qd + s%16.  Sign of sin is folded in:
    # even slots get -sin (r1 = x1 c - x2 s), odd slots +sin.
    r = np.arange(128)
    w = r % 64
    qd, s = w // 32, w % 32
    half, f = s // 16, 16 * qd + (s % 16)
    cct = cos.T[f, :]                                   # [128, T]
    sst = np.where((half == 0)[:, None], -sin.T[f, :], sin.T[f, :])
    # pre-permute sin rows by the shuffle partner p -> p^16 (per quadrant):
    # u[p] = q[p]*sst_pre[p], shuffle(u)[p] = q[p^16]*sst[p]
    partner = (r // 32) * 32 + ((r % 32) ^ 16)
    sst = sst[partner]
    # mask pair for the diagonal-block PE matmul: st += negI^T @ triU
    triu = (np.arange(128)[None, :] < np.arange(128)[:, None])   # [j, i] j>i
    negi = np.eye(128, dtype=np.float32) * np.float32(-60000.0)
    msk = np.concatenate([triu.astype(np.float32), negi], axis=1)

    x_b = [np.ascontiguousarray(x[b].T).astype(BF) for b in range(B)]

    wq = w_qkv[:, 0:D]
    wk = w_qkv[:, D:2 * D] * np.float32(1.0 / np.sqrt(HD))
    wv = w_qkv[:, 2 * D:3 * D]

    def build_qk_aug(g):
        # per head: 64 columns ordered [qd][half][j] -> original col 2f+half
        cols = []
        for wm in (wq, wk):
            for pr in range(4):
                for hl in (0, 1):
                    hw = wm[:, (g * 8 + 2 * pr + hl) * 64:
                            (g * 8 + 2 * pr + hl + 1) * 64]
                    cols.append(hw[:, (2 * f + half)[:64]])
        return np.concatenate(cols, axis=1)             # [D, 1024]

    wqk_g = []
    for g in range(2):
        wa = build_qk_aug(g)                            # [1024(rows), 1024]
        # device layout [128, ft, kt, 128]: wqk[p, ft, kt, c] =
        # wa[kt*128+p, ft*128+c] -> per-ft slices are contiguous 2KB runs
        wqk_g.append(np.ascontiguousarray(
            wa.reshape(NKT, 128, 8, 128).transpose(1, 2, 0, 3)).astype(BF))
    wv_g = [np.ascontiguousarray(
        wv[:, g * 512:(g + 1) * 512].reshape(NKT, 128, 512)
        .transpose(1, 0, 2)).astype(BF) for g in range(2)]
    wp_g = [np.ascontiguousarray(
        w_proj[g * 512:(g + 1) * 512, :].reshape(4, 128, D)
        .transpose(1, 0, 2)).astype(BF) for g in range(2)]

    in_maps = []
    for c in range(N_CORES):
        b, g = c // 2, c % 2
        in_maps.append({
            "x": x_b[b], "wqk": wqk_g[g], "wv": wv_g[g], "wproj": wp_g[g],
            "cc": cct.astype(BF), "ss": sst.astype(BF),
            "msk": msk.astype(BF),
        })
    return in_maps


def kernel(x, cos, sin, w_qkv, w_proj):
    nc = _get_program()
    in_maps = _host_inputs(x, cos, sin, w_qkv, w_proj)
    res = run_bass_kernel_spmd(nc, in_maps, core_ids=list(range(N_CORES)))
    out = np.empty((B, T, D), dtype=np.float32)
    for b in range(B):
        out[b] = (res.results[2 * b]["out"].astype(np.float32)
                  + res.results[2 * b + 1]["out"].astype(np.float32))
    return out
